# revision 10
# baseline (speedup 1.0000x reference)
"""DemonsOrientation loss kernel for Trainium2 (8 NeuronCores).

Math (reference): six separable 3x3x3 Sobel-style gradients of M and S,
demons orientation angles arctan(Ux/Uz), arctan(Uy/Uz), flow orientation
angles, and the mean of squared angle differences.

Decomposition per gradient (cross-correlation, padding=1):
  kx = box_d  (x) smooth_h (x) diff_w
  ky = box_d  (x) diff_h   (x) smooth_w
  kz = diff_d (x) smooth_h (x) box_w
with box = [1,1,1], smooth = [1,2,1], diff = [-1,0,1].

Sharding: D=160 split 8 ways (20 slices/core + 1-slice halo, sliced from
the full input on the host; no device-side exchange needed). Within a
core, H=192 splits into chunk A (input h 0..127 -> output h 0..126) and
chunk B (input h 126..191 -> output h 127..191).

Per core:
  TensorE (bf16): banded matmuls apply the h-stencil; PSUM accumulation
    over three d-shifted rhs views applies the d-stencil. Per input X in
    {S, M}: P1 = smooth_h(box_d X), P2 = diff_h(box_d X),
    P3 = smooth_h(diff_d X), packed in one 3-bank PSUM tile. An identity
    band computes Idiff = M - S in PSUM as well.
  ScalarE: evacuates P* to SBUF bf16 (plus a w-shifted copy of P2/P3 so
    every w-stencil op is 4B-aligned for the DVE 2x bf16 mode); squares;
    arctans; error squares with accum_out per-partition partial sums.
  DVE/GPSIMD: w-stencils and the bf16 pointwise chain. With
    q = denom_S/denom_M: t_xz = (Sx + q Mx)/(Sz + q Mz + eps),
    algebraically equal to the reference Ux/(Uz+1e-10) up to the
    vanishing stabilizer. x/y components ride through ops pairwise;
    q/rnz/rfz are broadcast via step-0 APs. The approx-reciprocal custom
    DVE op needs fp32, so denom_M, Sz+q*Mz and fz+eps are fp32, with
    epsilons added after the cancelling sums (exact-zero inputs NaN the
    reciprocal).
  Host: sums the per-partition fp32 accumulators (fp64) / voxel count.

bf16 end-to-end rel-err vs the fp32 reference: ~1e-4 (per-voxel rounding
noise averages out over the 4.9M-voxel mean).
"""

import numpy as np
import ml_dtypes
from contextlib import ExitStack

import concourse.bass as bass
import concourse.bacc as bacc
import concourse.tile as tile
from concourse import mybir
from concourse.bass_utils import run_bass_kernel_spmd

F32 = mybir.dt.float32
BF16 = mybir.dt.bfloat16
NPBF = ml_dtypes.bfloat16

D, H, W = 160, 192, 160
NCORES = 8
DL = D // NCORES          # 20 out slices per core
DS = DL + 2               # slab d extent (with halo)
WH = W + 2                # w extent with halo
G = 2                     # d-slices per stencil round (PSUM bank limit)
G2 = 4                    # d-slices per pointwise round
NPW = DL // G2            # pointwise rounds
RPP = G2 // G             # stencil rounds per pointwise round
NW = G * WH               # free size of one P* block per round (324)

AIN, AOUT = 128, 127      # chunk A: input h 0..127 -> out h 0..126
BIN, BOUT = 66, 65        # chunk B: input h 126..191 -> out h 127..191

EPS = 1e-10
NACC = NPW


def _band_matrices():
    """lhsT band matrices for the h-stencil matmuls (out = lhsT.T @ rhs)."""
    def mk(pin, pout, off, taps):
        b = np.zeros((pin, pout), np.float32)
        for m in range(pout):
            for dk, c in taps:
                k = m + off + dk
                if 0 <= k < pin:
                    b[k, m] = c
        return b
    sm = ((-1, 1.0), (0, 2.0), (1, 1.0))
    df = ((-1, -1.0), (1, 1.0))
    ident = ((0, 1.0),)
    out = {
        "gBsA": mk(AIN, AOUT, 0, sm), "gBdA": mk(AIN, AOUT, 0, df),
        "gIdA": mk(AIN, AOUT, 0, ident),
        "gBsB": mk(BIN, BOUT, 1, sm), "gBdB": mk(BIN, BOUT, 1, df),
        "gIdB": mk(BIN, BOUT, 1, ident),
    }
    out["gBnA"] = -out["gBsA"]
    out["gBnB"] = -out["gBsB"]
    out["gInA"] = -out["gIdA"]
    out["gInB"] = -out["gIdB"]
    return out


_BANDS = ("gBsA", "gBdA", "gBnA", "gIdA", "gInA",
          "gBsB", "gBdB", "gBnB", "gIdB", "gInB")


def _build_nc():
    nc = bacc.Bacc("TRN2")
    din = {}
    for nm, shp in (("gMA", [AIN, DS, WH]), ("gMB", [BIN, DS, WH]),
                    ("gSA", [AIN, DS, WH]), ("gSB", [BIN, DS, WH]),
                    ("gFA", [AOUT, 3, DL, W]), ("gFB", [BOUT, 3, DL, W])):
        din[nm] = nc.dram_tensor(nm, shp, BF16, kind="ExternalInput")
    for nm in _BANDS:
        pin, pout = (AIN, AOUT) if nm.endswith("A") else (BIN, BOUT)
        din[nm] = nc.dram_tensor(nm, [pin, pout], BF16, kind="ExternalInput")
    accA = nc.dram_tensor("gaccA", [AOUT, NACC], F32, kind="ExternalOutput")
    accB = nc.dram_tensor("gaccB", [BOUT, NACC], F32, kind="ExternalOutput")

    AL = mybir.AluOpType
    AF = mybir.ActivationFunctionType

    def bcast2(ap):
        """[P, G2, W] -> [P, 2, G2, W] with step-0 leading free dim."""
        return bass.AP(tensor=ap.tensor, offset=ap.offset,
                       ap=[ap.ap[0], [0, 2]] + list(ap.ap[1:]))

    with ExitStack() as ctx:
        tc = ctx.enter_context(tile.TileContext(nc))
        persist = ctx.enter_context(tc.tile_pool(name="persist", bufs=1))
        psum = ctx.enter_context(tc.tile_pool(name="psum", bufs=1, space="PSUM"))
        gpool = ctx.enter_context(tc.tile_pool(name="gpool", bufs=2))
        fpool = ctx.enter_context(tc.tile_pool(name="fpool", bufs=3))
        cpool = ctx.enter_context(tc.tile_pool(name="cpool", bufs=2))
        wpool = ctx.enter_context(tc.tile_pool(name="wpool", bufs=4))
        vpool = ctx.enter_context(tc.tile_pool(name="vpool", bufs=18))

        slab = {}
        for nm, pin in (("gMA", AIN), ("gMB", BIN), ("gSA", AIN), ("gSB", BIN)):
            st = persist.tile([pin, DS, WH], BF16, tag=nm, name=nm)
            nc.sync.dma_start(out=st, in_=din[nm][:, :, :])
            slab[nm] = st
        band = {}
        for nm in _BANDS:
            pin, pout = (AIN, AOUT) if nm.endswith("A") else (BIN, BOUT)
            bt = persist.tile([pin, pout], BF16, tag=nm, name=nm)
            nc.sync.dma_start(out=bt, in_=din[nm][:, :])
            band[nm] = bt
        acc_t = {
            "A": persist.tile([AOUT, NACC], F32, tag="accAt", name="accAt"),
            "B": persist.tile([BOUT, NACC], F32, tag="accBt", name="accBt"),
        }

        for pw in range(NPW):
            for ch in ("A", "B"):
                pout = AOUT if ch == "A" else BOUT
                Bs, Bd, Bn = band["gBs" + ch], band["gBd" + ch], band["gBn" + ch]
                Id, In = band["gId" + ch], band["gIn" + ch]
                msl, ssl = slab["gM" + ch], slab["gS" + ch]

                # c: evacuated P1..P3 [pout, 3, G2, WH]; co: w+1-shifted
                # copies of P2, P3 [pout, 2, G2, WH-1]; i2 from the PE Idiff
                ct, cot = {}, {}
                for Xn in ("S", "M"):
                    ct[Xn] = cpool.tile([pout, 3, G2, WH], BF16,
                                        tag="c" + Xn, name="c" + Xn)
                    cot[Xn] = cpool.tile([pout, 2, G2, WH], BF16,
                                         tag="co" + Xn, name="co" + Xn)
                i2 = vpool.tile([pout, G2, W], BF16, tag="v", name="i2")

                for rr in range(RPP):
                    s0 = (pw * RPP + rr) * G
                    dsl = slice(rr * G, rr * G + G)
                    for Xn, xsl in (("S", ssl), ("M", msl)):
                        # pt: P1|P2|P3 in 3 bank-aligned 512-col blocks
                        pt = psum.tile([pout, 3, G, 256], F32, tag="p" + Xn,
                                       name="p" + Xn)
                        for i in (0, 1, 2):
                            rhs = xsl[:, s0 + i: s0 + i + G, :]
                            nc.tensor.matmul(pt[:, 0, :, 0:WH], Bs, rhs,
                                             start=(i == 0), stop=(i == 2))
                        for i in (0, 1, 2):
                            rhs = xsl[:, s0 + i: s0 + i + G, :]
                            nc.tensor.matmul(pt[:, 1, :, 0:WH], Bd, rhs,
                                             start=(i == 0), stop=(i == 2))
                        nc.tensor.matmul(pt[:, 2, :, 0:WH], Bs,
                                         xsl[:, s0 + 2: s0 + 2 + G, :],
                                         start=True, stop=False)
                        nc.tensor.matmul(pt[:, 2, :, 0:WH], Bn,
                                         xsl[:, s0: s0 + G, :],
                                         start=False, stop=True)
                        # single evacuation per X-round + odd-tap copy
                        nc.scalar.copy(ct[Xn][:, :, dsl, :], pt[:, :, :, 0:WH])
                        nc.scalar.copy(cot[Xn][:, :, dsl, 0:WH - 1],
                                       pt[:, 1:3, :, 1:WH])
                    # Idiff = M - S via identity bands (bank-aligned block)
                    pi = psum.tile([pout, G, 256], F32, tag="pI", name="pI")
                    nc.tensor.matmul(pi[:, :, 0:WH], Id,
                                     msl[:, s0 + 1: s0 + 1 + G, :],
                                     start=True, stop=False)
                    nc.tensor.matmul(pi[:, :, 0:WH], In,
                                     ssl[:, s0 + 1: s0 + 1 + G, :],
                                     start=False, stop=True)
                    nc.scalar.activation(i2[:, dsl, :], pi[:, :, 1:W + 1],
                                         AF.Square)

                # w stencils (all operands 4B-aligned -> DVE bf16 2x)
                gt = {}
                for Xn in ("S", "M"):
                    c, co = ct[Xn], cot[Xn]
                    g = gpool.tile([pout, 3, G2, W], BF16, tag="g" + Xn,
                                   name="g" + Xn)
                    gt[Xn] = g
                    # Gx = P1[j+2] - P1[j]
                    nc.vector.tensor_sub(g[:, 0], c[:, 0, :, 2:WH],
                                         c[:, 0, :, 0:W])
                    # Gy = (2*P2[j+1] + P2[j]) + P2[j+2]
                    t1 = wpool.tile([pout, G2, W], BF16, tag="wt1", name="wt1")
                    nc.vector.scalar_tensor_tensor(
                        t1, co[:, 0, :, 0:W], 2.0, c[:, 1, :, 0:W],
                        op0=AL.mult, op1=AL.add)
                    nc.vector.tensor_add(g[:, 1], t1, c[:, 1, :, 2:WH])
                    # Gz = (P3[j] + P3[j+1]) + P3[j+2]
                    t2 = wpool.tile([pout, G2, W], BF16, tag="wt2", name="wt2")
                    nc.gpsimd.tensor_add(t2, co[:, 1, :, 0:W], c[:, 2, :, 0:W])
                    nc.gpsimd.tensor_add(g[:, 2], t2, c[:, 2, :, 2:WH])

                # ---- pointwise ----
                def vt(tag, dt=BF16, two=False):
                    shp = [pout, 2, G2, W] if two else [pout, G2, W]
                    return vpool.tile(shp, dt, tag="v", name=tag)

                gS, gM = gt["S"], gt["M"]
                sqS = vpool.tile([pout, 3, G2, W], BF16, tag="v", name="sqS")
                sqM = vpool.tile([pout, 3, G2, W], BF16, tag="v", name="sqM")
                nc.scalar.activation(sqS, gS, AF.Square)
                nc.scalar.activation(sqM, gM, AF.Square)
                dS0 = vt("dS0")
                nc.gpsimd.tensor_add(dS0, sqS[:, 0], sqS[:, 1])
                dS1 = vt("dS1")
                nc.vector.tensor_add(dS1, dS0, sqS[:, 2])
                dS = vt("dS")
                nc.vector.scalar_tensor_tensor(dS, i2, EPS, dS1,
                                               op0=AL.add, op1=AL.add)
                dM0 = vt("dM0")
                nc.gpsimd.tensor_add(dM0, sqM[:, 0], sqM[:, 1])
                dM1 = vt("dM1")
                nc.gpsimd.tensor_add(dM1, dM0, sqM[:, 2])
                dM = vt("dM", F32)
                nc.vector.scalar_tensor_tensor(dM, i2, EPS, dM1,
                                               op0=AL.add, op1=AL.add)
                rdM = vt("rdM", F32)
                nc.vector.reciprocal_approx_fast(rdM, dM)
                q = vt("q")
                nc.gpsimd.tensor_mul(q, dS, rdM)

                m12 = vt("m12", two=True)
                nc.vector.tensor_mul(m12, gM[:, 0:2], bcast2(q))
                n12 = vt("n12", two=True)
                nc.vector.tensor_add(n12, m12, gS[:, 0:2])
                mz = vt("mz")
                nc.gpsimd.tensor_mul(mz, gM[:, 2], q)
                nz0 = vt("nz0", F32)
                nc.vector.tensor_add(nz0, mz, gS[:, 2])
                nz = vt("nz", F32)
                nc.vector.tensor_scalar_add(nz, nz0, 1e-12)
                rnz = vt("rnz", F32)
                nc.vector.reciprocal_approx_fast(rnz, nz)
                t12 = vt("t12", two=True)
                nc.vector.tensor_mul(t12, n12, bcast2(rnz))
                a12 = vt("a12", two=True)
                nc.scalar.activation(a12, t12, AF.Arctan)

                # flow side
                ft = fpool.tile([pout, 3, G2, W], BF16, tag="flow", name="flow")
                fdr = din["gFA"] if ch == "A" else din["gFB"]
                d0 = pw * G2
                nc.sync.dma_start(out=ft, in_=fdr[:, :, d0: d0 + G2, :])
                fze = vt("fze", F32)
                nc.vector.tensor_scalar_add(fze, ft[:, 2], EPS)
                rfz = vt("rfz", F32)
                nc.vector.reciprocal_approx_fast(rfz, fze)
                t34 = vt("t34", two=True)
                nc.gpsimd.tensor_mul(t34, ft[:, 0:2], bcast2(rfz))
                b12 = vt("b12", two=True)
                nc.scalar.activation(b12, t34, AF.Arctan)

                # error accumulation (both components in one accum)
                d12 = vt("d12", two=True)
                nc.vector.tensor_sub(d12, b12, a12)
                scr = vt("scr", two=True)
                nc.scalar.activation(scr, d12, AF.Square,
                                     accum_out=acc_t[ch][:, pw: pw + 1])

        nc.sync.dma_start(out=accA[:, :], in_=acc_t["A"])
        nc.sync.dma_start(out=accB[:, :], in_=acc_t["B"])

    nc.compile()
    return nc


_NC_CACHE = None


def _get_nc():
    global _NC_CACHE
    if _NC_CACHE is None:
        _NC_CACHE = _build_nc()
    return _NC_CACHE


def _prep_inputs(M, S, flow):
    M3 = np.asarray(M, np.float32).reshape(D, H, W)
    S3 = np.asarray(S, np.float32).reshape(D, H, W)
    F3 = np.asarray(flow, np.float32).reshape(3, D, H, W)
    MP = np.zeros((D + 2, H, WH), NPBF)
    SP = np.zeros((D + 2, H, WH), NPBF)
    MP[1:D + 1, :, 1:W + 1] = M3.astype(NPBF)
    SP[1:D + 1, :, 1:W + 1] = S3.astype(NPBF)
    FBF = F3.astype(NPBF)
    bands = {k: v.astype(NPBF) for k, v in _band_matrices().items()}
    in_maps = []
    for c in range(NCORES):
        msl = np.ascontiguousarray(MP[c * DL: c * DL + DS].transpose(1, 0, 2))
        ssl = np.ascontiguousarray(SP[c * DL: c * DL + DS].transpose(1, 0, 2))
        fsl = np.ascontiguousarray(
            FBF[:, c * DL: (c + 1) * DL].transpose(2, 0, 1, 3))
        in_maps.append({
            "gMA": msl[0:AIN], "gMB": np.ascontiguousarray(msl[H - BIN: H]),
            "gSA": ssl[0:AIN], "gSB": np.ascontiguousarray(ssl[H - BIN: H]),
            "gFA": fsl[0:AOUT], "gFB": np.ascontiguousarray(fsl[H - BOUT: H]),
            **bands,
        })
    return in_maps


def kernel(M, S, flow):
    nc = _get_nc()
    in_maps = _prep_inputs(M, S, flow)
    res = run_bass_kernel_spmd(nc, in_maps, core_ids=list(range(NCORES)))
    tot = 0.0
    for r in res.results:
        tot += r["gaccA"].astype(np.float64).sum()
        tot += r["gaccB"].astype(np.float64).sum()
    return np.float32(tot / (D * H * W))


# revision 12
# speedup vs baseline: 1.2071x; 1.2071x over previous
"""DemonsOrientation loss kernel for Trainium2 (8 NeuronCores).

Math (reference): six separable 3x3x3 Sobel-style gradients of M and S,
demons orientation angles arctan(Ux/Uz), arctan(Uy/Uz), flow orientation
angles, and the mean of squared angle differences.

Decomposition per gradient (cross-correlation, padding=1):
  kx = box_d  (x) smooth_h (x) diff_w
  ky = box_d  (x) diff_h   (x) smooth_w
  kz = diff_d (x) smooth_h (x) box_w
with box = [1,1,1], smooth = [1,2,1], diff = [-1,0,1].

Sharding: D=160 split 8 ways (20 slices/core + 1-slice halo, sliced from
the full input on the host; no device-side exchange needed). Within a
core, H=192 splits into chunk A (input h 0..127 -> output h 0..126) and
chunk B (input h 126..191 -> output h 127..191).

Per core:
  TensorE (bf16): banded matmuls apply the h-stencil; PSUM accumulation
    over three d-shifted rhs views applies the d-stencil. Per input X in
    {S, M}: P1 = smooth_h(box_d X), P2 = diff_h(box_d X),
    P3 = smooth_h(diff_d X), packed in one 3-bank PSUM tile. An identity
    band computes Idiff = M - S in PSUM as well.
  ScalarE: evacuates P* to SBUF bf16 (plus a w-shifted copy of P2/P3 so
    every w-stencil op is 4B-aligned for the DVE 2x bf16 mode); squares;
    arctans; error squares with accum_out per-partition partial sums.
  DVE/GPSIMD: w-stencils and the bf16 pointwise chain. With
    q = denom_S/denom_M: t_xz = (Sx + q Mx)/(Sz + q Mz + eps),
    algebraically equal to the reference Ux/(Uz+1e-10) up to the
    vanishing stabilizer. x/y components ride through ops pairwise;
    q/rnz/rfz are broadcast via step-0 APs. The approx-reciprocal custom
    DVE op needs fp32, so denom_M, Sz+q*Mz and fz+eps are fp32, with
    epsilons added after the cancelling sums (exact-zero inputs NaN the
    reciprocal).
  Host: sums the per-partition fp32 accumulators (fp64) / voxel count.

bf16 end-to-end rel-err vs the fp32 reference: ~1e-4 (per-voxel rounding
noise averages out over the 4.9M-voxel mean).
"""

import numpy as np
import ml_dtypes
from contextlib import ExitStack

import concourse.bass as bass
import concourse.bacc as bacc
import concourse.tile as tile
from concourse import mybir
from concourse.bass_utils import run_bass_kernel_spmd

F32 = mybir.dt.float32
BF16 = mybir.dt.bfloat16
NPBF = ml_dtypes.bfloat16

D, H, W = 160, 192, 160
NCORES = 8
DL = D // NCORES          # 20 out slices per core
DS = DL + 2               # slab d extent (with halo)
WH = W + 2                # w extent with halo
G = 2                     # d-slices per stencil round (PSUM bank limit)
G2 = 4                    # d-slices per pointwise round
NPW = DL // G2            # pointwise rounds
RPP = G2 // G             # stencil rounds per pointwise round
NW = G * WH               # free size of one P* block per round (324)

AIN, AOUT = 128, 127      # chunk A: input h 0..127 -> out h 0..126
BIN, BOUT = 66, 65        # chunk B: input h 126..191 -> out h 127..191

EPS = 1e-10
NACC = NPW


def _band_matrices():
    """lhsT band matrices for the h-stencil matmuls (out = lhsT.T @ rhs)."""
    def mk(pin, pout, off, taps):
        b = np.zeros((pin, pout), np.float32)
        for m in range(pout):
            for dk, c in taps:
                k = m + off + dk
                if 0 <= k < pin:
                    b[k, m] = c
        return b
    sm = ((-1, 1.0), (0, 2.0), (1, 1.0))
    df = ((-1, -1.0), (1, 1.0))
    ident = ((0, 1.0),)
    out = {
        "gBsA": mk(AIN, AOUT, 0, sm), "gBdA": mk(AIN, AOUT, 0, df),
        "gIdA": mk(AIN, AOUT, 0, ident),
        "gBsB": mk(BIN, BOUT, 1, sm), "gBdB": mk(BIN, BOUT, 1, df),
        "gIdB": mk(BIN, BOUT, 1, ident),
    }
    out["gBnA"] = -out["gBsA"]
    out["gBnB"] = -out["gBsB"]
    out["gInA"] = -out["gIdA"]
    out["gInB"] = -out["gIdB"]
    return out


_BANDS = ("gBsA", "gBdA", "gBnA", "gIdA", "gInA",
          "gBsB", "gBdB", "gBnB", "gIdB", "gInB")


def _build_nc():
    nc = bacc.Bacc("TRN2")
    din = {}
    for nm, shp in (("gMA", [AIN, DS, WH]), ("gMB", [BIN, DS, WH]),
                    ("gSA", [AIN, DS, WH]), ("gSB", [BIN, DS, WH]),
                    ("gFA", [AOUT, 3, DL, W]), ("gFB", [BOUT, 3, DL, W])):
        din[nm] = nc.dram_tensor(nm, shp, BF16, kind="ExternalInput")
    for nm in _BANDS:
        pin, pout = (AIN, AOUT) if nm.endswith("A") else (BIN, BOUT)
        din[nm] = nc.dram_tensor(nm, [pin, pout], BF16, kind="ExternalInput")
    accA = nc.dram_tensor("gaccA", [AOUT, NACC], F32, kind="ExternalOutput")
    accB = nc.dram_tensor("gaccB", [BOUT, NACC], F32, kind="ExternalOutput")

    AL = mybir.AluOpType
    AF = mybir.ActivationFunctionType

    def bcast2(ap):
        """[P, G2, W] -> [P, 2, G2, W] with step-0 leading free dim."""
        return bass.AP(tensor=ap.tensor, offset=ap.offset,
                       ap=[ap.ap[0], [0, 2]] + list(ap.ap[1:]))

    with ExitStack() as ctx:
        tc = ctx.enter_context(tile.TileContext(nc))
        persist = ctx.enter_context(tc.tile_pool(name="persist", bufs=1))
        psum = ctx.enter_context(tc.tile_pool(name="psum", bufs=1, space="PSUM"))
        gpool = ctx.enter_context(tc.tile_pool(name="gpool", bufs=2))
        fpool = ctx.enter_context(tc.tile_pool(name="fpool", bufs=3))
        cpool = ctx.enter_context(tc.tile_pool(name="cpool", bufs=2))
        wpool = ctx.enter_context(tc.tile_pool(name="wpool", bufs=4))
        vpool = ctx.enter_context(tc.tile_pool(name="vpool", bufs=22))

        slab = {}
        for nm, pin in (("gMA", AIN), ("gMB", BIN), ("gSA", AIN), ("gSB", BIN)):
            st = persist.tile([pin, DS, WH], BF16, tag=nm, name=nm)
            nc.sync.dma_start(out=st, in_=din[nm][:, :, :])
            slab[nm] = st
        band = {}
        for nm in _BANDS:
            pin, pout = (AIN, AOUT) if nm.endswith("A") else (BIN, BOUT)
            bt = persist.tile([pin, pout], BF16, tag=nm, name=nm)
            nc.sync.dma_start(out=bt, in_=din[nm][:, :])
            band[nm] = bt
        acc_t = {
            "A": persist.tile([AOUT, NACC], F32, tag="accAt", name="accAt"),
            "B": persist.tile([BOUT, NACC], F32, tag="accBt", name="accBt"),
        }

        for pw in range(NPW):
            for ch in ("A", "B"):
                pout = AOUT if ch == "A" else BOUT
                Bs, Bd, Bn = band["gBs" + ch], band["gBd" + ch], band["gBn" + ch]
                Id, In = band["gId" + ch], band["gIn" + ch]
                msl, ssl = slab["gM" + ch], slab["gS" + ch]

                # c: evacuated P1..P3 [pout, 3, G2, WH]; co: w+1-shifted
                # copies of P2, P3 [pout, 2, G2, WH-1]; i2 from the PE Idiff
                ct, cot = {}, {}
                for Xn in ("S", "M"):
                    ct[Xn] = cpool.tile([pout, 3, G2, WH], BF16,
                                        tag="c" + Xn, name="c" + Xn)
                    cot[Xn] = cpool.tile([pout, 2, G2, WH], BF16,
                                         tag="co" + Xn, name="co" + Xn)
                i2 = vpool.tile([pout, G2, W], BF16, tag="v", name="i2")

                for rr in range(RPP):
                    s0 = (pw * RPP + rr) * G
                    dsl = slice(rr * G, rr * G + G)
                    for Xn, xsl in (("S", ssl), ("M", msl)):
                        # pt: P1|P2|P3 in 3 bank-aligned 512-col blocks
                        pt = psum.tile([pout, 3, G, 256], F32, tag="pX",
                                       bufs=2, name="p" + Xn)
                        for i in (0, 1, 2):
                            rhs = xsl[:, s0 + i: s0 + i + G, :]
                            nc.tensor.matmul(pt[:, 0, :, 0:WH], Bs, rhs,
                                             start=(i == 0), stop=(i == 2))
                        for i in (0, 1, 2):
                            rhs = xsl[:, s0 + i: s0 + i + G, :]
                            nc.tensor.matmul(pt[:, 1, :, 0:WH], Bd, rhs,
                                             start=(i == 0), stop=(i == 2))
                        nc.tensor.matmul(pt[:, 2, :, 0:WH], Bs,
                                         xsl[:, s0 + 2: s0 + 2 + G, :],
                                         start=True, stop=False)
                        nc.tensor.matmul(pt[:, 2, :, 0:WH], Bn,
                                         xsl[:, s0: s0 + G, :],
                                         start=False, stop=True)
                        # single evacuation per X-round + scaled odd-tap
                        # copies (2*P2[w+1] and 1*P3[w+1])
                        nc.scalar.copy(ct[Xn][:, :, dsl, :], pt[:, :, :, 0:WH])
                        nc.scalar.mul(cot[Xn][:, 0, dsl, 0:WH - 1],
                                      pt[:, 1, :, 1:WH], 2.0)
                        nc.scalar.copy(cot[Xn][:, 1, dsl, 0:WH - 1],
                                       pt[:, 2, :, 1:WH])
                    # Idiff = M - S via identity bands (bank-aligned block)
                    pi = psum.tile([pout, G, 256], F32, tag="pI", name="pI")
                    nc.tensor.matmul(pi[:, :, 0:WH], Id,
                                     msl[:, s0 + 1: s0 + 1 + G, :],
                                     start=True, stop=False)
                    nc.tensor.matmul(pi[:, :, 0:WH], In,
                                     ssl[:, s0 + 1: s0 + 1 + G, :],
                                     start=False, stop=True)
                    nc.scalar.activation(i2[:, dsl, :], pi[:, :, 1:W + 1],
                                         AF.Square)

                # w stencils (all operands 4B-aligned -> DVE bf16 2x)
                gt = {}
                for Xn in ("S", "M"):
                    c, co = ct[Xn], cot[Xn]
                    g = gpool.tile([pout, 3, G2, W], BF16, tag="g" + Xn,
                                   name="g" + Xn)
                    gt[Xn] = g
                    # Gx = P1[j+2] - P1[j]
                    nc.vector.tensor_sub(g[:, 0], c[:, 0, :, 2:WH],
                                         c[:, 0, :, 0:W])
                    # Gy = (2*P2[j+1] + P2[j]) + P2[j+2]
                    # Gz = (1*P3[j+1] + P3[j]) + P3[j+2]   (scale in co)
                    t1 = wpool.tile([pout, 2, G2, W], BF16, tag="wt1", name="wt1")
                    nc.vector.tensor_add(t1, co[:, :, :, 0:W],
                                         c[:, 1:3, :, 0:W])
                    nc.vector.tensor_add(g[:, 1:3], t1, c[:, 1:3, :, 2:WH])

                # ---- pointwise ----
                def vt(tag, dt=BF16, two=False):
                    shp = [pout, 2, G2, W] if two else [pout, G2, W]
                    return vpool.tile(shp, dt, tag="v", name=tag)

                gS, gM = gt["S"], gt["M"]
                sqS = vpool.tile([pout, 3, G2, W], BF16, tag="v", name="sqS")
                sqM = vpool.tile([pout, 3, G2, W], BF16, tag="v", name="sqM")
                nc.scalar.activation(sqS, gS, AF.Square)
                nc.scalar.activation(sqM, gM, AF.Square)
                dS0 = vt("dS0")
                nc.gpsimd.tensor_add(dS0, sqS[:, 0], sqS[:, 1])
                dS1 = vt("dS1")
                nc.vector.tensor_add(dS1, dS0, sqS[:, 2])
                dS = vt("dS")
                nc.vector.scalar_tensor_tensor(dS, i2, EPS, dS1,
                                               op0=AL.add, op1=AL.add)
                dM0 = vt("dM0")
                nc.gpsimd.tensor_add(dM0, sqM[:, 0], sqM[:, 1])
                dM1 = vt("dM1")
                nc.gpsimd.tensor_add(dM1, dM0, sqM[:, 2])
                dM = vt("dM")
                nc.vector.scalar_tensor_tensor(dM, i2, EPS, dM1,
                                               op0=AL.add, op1=AL.add)

                # N-form: Nc = Sc*dM + Mc*dS  (no reciprocal of dM needed;
                # t = Nx/(Nz+eps) equals the q-form ratio exactly)
                u12 = vt("u12", two=True)
                nc.vector.tensor_mul(u12, gS[:, 0:2], bcast2(dM))
                v12 = vt("v12", two=True)
                nc.gpsimd.tensor_mul(v12, gM[:, 0:2], bcast2(dS))
                n12 = vt("n12", two=True)
                nc.vector.tensor_add(n12, u12, v12)
                uz = vt("uz")
                nc.vector.tensor_mul(uz, gS[:, 2], dM)
                vz = vt("vz")
                nc.gpsimd.tensor_mul(vz, gM[:, 2], dS)
                nz0 = vt("nz0", F32)
                nc.vector.tensor_add(nz0, uz, vz)
                nz = vt("nz", F32)
                nc.vector.tensor_scalar_add(nz, nz0, 1e-12)
                rnz = vt("rnz", F32)
                nc.vector.reciprocal_approx_fast(rnz, nz)
                rnzb = vt("rnzb")
                nc.gpsimd.tensor_copy(rnzb, rnz)
                t12 = vt("t12", two=True)
                nc.vector.tensor_mul(t12, n12, bcast2(rnzb))
                a12 = vt("a12", two=True)
                nc.scalar.activation(a12, t12, AF.Arctan)

                # flow side
                ft = fpool.tile([pout, 3, G2, W], BF16, tag="flow", name="flow")
                fdr = din["gFA"] if ch == "A" else din["gFB"]
                d0 = pw * G2
                nc.sync.dma_start(out=ft, in_=fdr[:, :, d0: d0 + G2, :])
                fze = vt("fze", F32)
                nc.vector.tensor_scalar_add(fze, ft[:, 2], EPS)
                rfz = vt("rfz", F32)
                nc.vector.reciprocal_approx_fast(rfz, fze)
                rfzb = vt("rfzb")
                nc.gpsimd.tensor_copy(rfzb, rfz)
                t34 = vt("t34", two=True)
                nc.vector.tensor_mul(t34, ft[:, 0:2], bcast2(rfzb))
                b12 = vt("b12", two=True)
                nc.scalar.activation(b12, t34, AF.Arctan)

                # error accumulation (both components in one accum)
                d12 = vt("d12", two=True)
                nc.vector.tensor_sub(d12, b12, a12)
                scr = vt("scr", two=True)
                nc.scalar.activation(scr, d12, AF.Square,
                                     accum_out=acc_t[ch][:, pw: pw + 1])

        nc.sync.dma_start(out=accA[:, :], in_=acc_t["A"])
        nc.sync.dma_start(out=accB[:, :], in_=acc_t["B"])

    nc.compile()
    return nc


_NC_CACHE = None


def _get_nc():
    global _NC_CACHE
    if _NC_CACHE is None:
        _NC_CACHE = _build_nc()
    return _NC_CACHE


def _prep_inputs(M, S, flow):
    M3 = np.asarray(M, np.float32).reshape(D, H, W)
    S3 = np.asarray(S, np.float32).reshape(D, H, W)
    F3 = np.asarray(flow, np.float32).reshape(3, D, H, W)
    MP = np.zeros((D + 2, H, WH), NPBF)
    SP = np.zeros((D + 2, H, WH), NPBF)
    MP[1:D + 1, :, 1:W + 1] = M3.astype(NPBF)
    SP[1:D + 1, :, 1:W + 1] = S3.astype(NPBF)
    FBF = F3.astype(NPBF)
    bands = {k: v.astype(NPBF) for k, v in _band_matrices().items()}
    in_maps = []
    for c in range(NCORES):
        msl = np.ascontiguousarray(MP[c * DL: c * DL + DS].transpose(1, 0, 2))
        ssl = np.ascontiguousarray(SP[c * DL: c * DL + DS].transpose(1, 0, 2))
        fsl = np.ascontiguousarray(
            FBF[:, c * DL: (c + 1) * DL].transpose(2, 0, 1, 3))
        in_maps.append({
            "gMA": msl[0:AIN], "gMB": np.ascontiguousarray(msl[H - BIN: H]),
            "gSA": ssl[0:AIN], "gSB": np.ascontiguousarray(ssl[H - BIN: H]),
            "gFA": fsl[0:AOUT], "gFB": np.ascontiguousarray(fsl[H - BOUT: H]),
            **bands,
        })
    return in_maps


def kernel(M, S, flow):
    nc = _get_nc()
    in_maps = _prep_inputs(M, S, flow)
    res = run_bass_kernel_spmd(nc, in_maps, core_ids=list(range(NCORES)))
    tot = 0.0
    for r in res.results:
        tot += r["gaccA"].astype(np.float64).sum()
        tot += r["gaccB"].astype(np.float64).sum()
    return np.float32(tot / (D * H * W))
